# revision 1
# baseline (speedup 1.0000x reference)
"""GroupOfGESNCell Trainium2 kernel.

Math (reference): 5 fixed-point iterations over G=4 groups:
    wiu = einsum('nf,ghf->gnh', X, W_ih)                     # [G,N,H]
    hx  <- tanh(wiu + L @ (hx @ W_hh_g^T))   per group       # N=8192, H=64
    out = concat_g(hx_g) -> [N, G*H=256]
The convergence early-exit (diff < 1e-5) never triggers for this input
regime (diffs stay O(100)); 5 unconditional iterations are exact.
Iteration 0 starts from hx=0, so it reduces to hx1 = tanh(wiu) with no
L-matmul: only 4 big matmuls of L are needed.

Distribution: row-shard L over 8 cores (1024 rows each). Each core
keeps its shard of L resident in SBUF (bf16, 16 MB) for ALL
iterations -- L is loaded from HBM exactly once. Per iteration each
core computes lin = hx @ Whh^T for its rows, AllGathers lin (bf16,
0.25 MB/rank, split in two gh-halves to pipeline against the PE),
then computes new-hx rows = L_rows @ lin_full + wiu, tanh.

Iteration 1 is collective-free: hx1 = tanh(X @ Wih^T) depends only on
X, so every core redundantly computes lin_1 for ALL nodes from the
full X (bf16). This hides the ~50us first-collective/ncfw warmup and
the kernel-entry barrier under iteration-1 compute; the first real
AllGather (for iteration 2) enters the steady-state pipeline.

On-device layout: hx is kept transposed (hxT [GH, n]) so every matmul
consumes natural layouts with zero on-device transposes:
  small mm: lin[n,gk] = hxT[gh,n].T @ blockdiag(WhhT)    (lhsT = hxT)
  big   mm: hxT_new[gh,n] = lin_full[m,gh].T @ LT[m,n]   (lhsT = lin)
The host pre-transposes L (per-shard) and X once; that is sharding
prep, not part of the 172-GFLOP iteration.
"""

import sys

import numpy as np
import ml_dtypes

sys.path.insert(0, "/opt/trn_rl_repo")

N, F, H, G = 8192, 128, 64, 4
GH = G * H  # 256
NCORES = 8
ROWS = N // NCORES  # 1024 rows of L / nodes per core
KT = N // 128  # 64 contraction tiles for the big matmul
JT = ROWS // 128  # 8 n-tiles per core
NITER = 5

_CACHE = {}


def _build_kernel():
    import concourse.mybir as mybir
    import concourse.tile as tile
    from concourse import bacc

    f32 = mybir.dt.float32
    bf16 = mybir.dt.bfloat16
    Tanh = mybir.ActivationFunctionType.Tanh

    nc = bacc.Bacc(
        "TRN2", target_bir_lowering=False, debug=False, num_devices=NCORES
    )

    # Per-core inputs (host-prepped):
    #  LT    [N, ROWS] bf16 : L[rows_c, :].T  (contraction dim first)
    #  XT    [F, ROWS] f32  : X[rows_c, :].T   (for the f32 wiu kept locally)
    #  XTF   [F, N]    bf16 : full X.T         (iteration-1 redundant path)
    #  WihT  [F, GH]   f32  ; WihTb same in bf16
    #  Wbd0/1 [128,128] bf16: blockdiag(Whh_{2h}^T, Whh_{2h+1}^T)
    # Output: hxT_out [GH, ROWS] f32 (host transposes + stacks)
    lt_d = nc.declare_dram_parameter("LT", [N, ROWS], bf16, isOutput=False)
    xt_d = nc.declare_dram_parameter("XT", [F, ROWS], f32, isOutput=False)
    xtf_d = nc.declare_dram_parameter("XTF", [F, N], bf16, isOutput=False)
    wih_d = nc.declare_dram_parameter("WihT", [F, GH], f32, isOutput=False)
    wihb_d = nc.declare_dram_parameter("WihTb", [F, GH], bf16, isOutput=False)
    wbd_d = [
        nc.declare_dram_parameter(f"Wbd{h}", [128, 128], bf16, isOutput=False)
        for h in range(2)
    ]
    out_d = nc.declare_dram_parameter("hxT_out", [GH, ROWS], f32, isOutput=True)

    # Collective bounce buffers for iterations 2..4, per gh-half.
    # half=0 carries lin cols 0:128 (groups 0,1), half=1 cols 128:256.
    cc_in = {
        (t, h): nc.dram_tensor(f"ccin_{t}_{h}", [ROWS, 128], bf16)
        for t in range(2, NITER)
        for h in range(2)
    }
    cc_out = {
        (t, h): nc.dram_tensor(f"ccout_{t}_{h}", [N, 128], bf16, addr_space="Shared")
        for t in range(2, NITER)
        for h in range(2)
    }
    warm_in = nc.dram_tensor("warm_in", [128, 16], mybir.dt.bfloat16)
    warm_out = nc.dram_tensor(
        "warm_out", [128 * NCORES, 16], mybir.dt.bfloat16, addr_space="Shared"
    )
    groups = [list(range(NCORES))]

    with tile.TileContext(nc) as tc:
        with (
            tc.tile_pool(name="lt", bufs=1) as lt_pool,
            tc.tile_pool(name="linf", bufs=1) as linf_pool,
            tc.tile_pool(name="hxt", bufs=4) as hxt_pool,
            tc.tile_pool(name="wiu", bufs=1) as wiu_pool,
            tc.tile_pool(name="consts", bufs=1) as const_pool,
            tc.tile_pool(name="xtf", bufs=2) as xtf_pool,
            tc.tile_pool(name="hx1f", bufs=2) as hx1f_pool,
            tc.tile_pool(name="stage", bufs=1) as stage_pool,
            tc.tile_pool(name="outs", bufs=1) as out_pool,
            tc.tile_pool(name="bigp", bufs=4, space="PSUM") as bigp_pool,
            tc.tile_pool(name="smallp", bufs=4, space="PSUM") as smallp_pool,
        ):
            # ---- constants / static loads (sync queue) ----
            # xt/wih first: the wiu matmuls are the first PE work
            xt_sb = const_pool.tile([F, ROWS], f32, tag="xt")
            nc.sync.dma_start(xt_sb[:], xt_d[:, :])
            wih_sb = const_pool.tile([F, GH], f32, tag="wih")
            nc.sync.dma_start(wih_sb[:], wih_d[:, :])
            wihb_sb = const_pool.tile([F, GH], bf16, tag="wihb")
            nc.sync.dma_start(wihb_sb[:], wihb_d[:, :])
            wbd_sb = [
                const_pool.tile([128, 128], bf16, tag=f"wbd{h}", name=f"wbd{h}")
                for h in range(2)
            ]
            for h in range(2):
                nc.sync.dma_start(wbd_sb[h][:], wbd_d[h][:, :])
            # dummy AllGather, hidden under the prologue: absorbs the
            # ncfw/SDMA warm-up so the first real AllGather runs at the
            # steady ~16us instead of ~35us.
            nc.gpsimd.collective_compute(
                "AllGather",
                mybir.AluOpType.bypass,
                replica_groups=groups,
                ins=[warm_in[:, :]],
                outs=[warm_out[:, :]],
            )
            # first two X.T chunks loaded BEFORE the LT triggers so the
            # iteration-1 prologue is never stuck behind 16 MB of L
            xtf_tiles = []
            for cc in range(2):
                t_ = xtf_pool.tile([128, 2048], bf16, tag="xtfc", name="xtfc")
                nc.sync.dma_start(t_[:], xtf_d[:, 2048 * cc : 2048 * cc + 2048])
                xtf_tiles.append(t_)

            # L-shard resident in SBUF: 16 tiles of [128, 4, ROWS] bf16
            # (tile i holds contraction k-tiles 4i..4i+3). Triggered on the
            # sync queue (nothing latency-critical queues behind them in the
            # prologue); the ACT queue stays free for the tanh chain.
            lt_view = lt_d.rearrange("(i k p) n -> p (i k) n", p=128, k=4)
            lt_sb = []
            for i in range(16):
                t_ = lt_pool.tile([128, 4, ROWS], bf16, tag=f"lt{i}", name=f"lt{i}")
                nc.sync.dma_start(t_[:], lt_view[:, 4 * i : 4 * i + 4, :])
                lt_sb.append(t_)

            def lt_slice(k, nh):
                return lt_sb[k // 4][:, k % 4, 512 * nh : 512 * nh + 512]

            # ---- local wiu = (X_c @ Wih^T)^T in f32, for the per-iter add
            wiu_sb = [
                wiu_pool.tile([128, ROWS], f32, tag=f"wiu{m}", name=f"wiu{m}")
                for m in range(2)
            ]
            for m in range(2):
                for nh in range(2):
                    ps = bigp_pool.tile([128, 512], f32, tag="big", name="bigps")
                    nc.tensor.matmul(
                        ps[:],
                        lhsT=wih_sb[:, 128 * m : 128 * m + 128],
                        rhs=xt_sb[:, 512 * nh : 512 * nh + 512],
                        start=True,
                        stop=True,
                    )
                    nc.scalar.copy(wiu_sb[m][:, 512 * nh : 512 * nh + 512], ps[:])

            # gathered lin, bf16: 8 tiles of [128, 8, 128] per gh-half
            # (one tile per DMA so each k-matmul waits on a single queue)
            linf = [
                [
                    linf_pool.tile(
                        [128, 8, 128], bf16, tag=f"linf{h}_{i}", name=f"linf{h}_{i}"
                    )
                    for i in range(8)
                ]
                for h in range(2)
            ]

            # ---- iteration-1 lin for ALL nodes, computed locally ----
            # hx1 = tanh(X @ Wih^T); lin1 = hx1 @ blockdiag -> linf direct.
            # X.T (bf16, full) streams through a rotating 2-buf pool.
            for cc in range(4):  # [128, 2048] chunks of full X.T
                if cc >= 2:
                    t_ = xtf_pool.tile([128, 2048], bf16, tag="xtfc", name="xtfc")
                    nc.sync.dma_start(t_[:], xtf_d[:, 2048 * cc : 2048 * cc + 2048])
                    xtf_tiles.append(t_)
                xtfc = xtf_tiles[cc]
                for m in range(2):
                    for s in range(4):  # 512-col sub-chunks
                        c = 4 * cc + s  # global 512-chunk 0..15
                        ps = bigp_pool.tile([128, 512], f32, tag="big", name="bigps")
                        nc.tensor.matmul(
                            ps[:],
                            lhsT=wihb_sb[:, 128 * m : 128 * m + 128],
                            rhs=xtfc[:, 512 * s : 512 * s + 512],
                            start=True,
                            stop=True,
                        )
                        hx1c = hx1f_pool.tile([128, 512], bf16, tag="hx1c", name="hx1c")
                        nc.scalar.activation(hx1c[:], ps[:], Tanh)
                        for j in range(4):
                            k = 4 * c + j  # global k-tile 0..63
                            ps2 = smallp_pool.tile(
                                [128, 128], f32, tag="small", name="smallps"
                            )
                            nc.tensor.matmul(
                                ps2[:],
                                lhsT=hx1c[:, 128 * j : 128 * j + 128],
                                rhs=wbd_sb[m][:],
                                start=True,
                                stop=True,
                            )
                            nc.vector.tensor_copy(linf[m][k // 8][:, k % 8, :], ps2[:])

            def small_mm_and_ag(t, h, hx_tile):
                """lin cols [128h:128h+128] for local rows from hx_tile,
                then AllGather into linf[h]."""
                stg = stage_pool.tile(
                    [128, JT, 128], bf16, tag=f"stg{h}", name=f"stg{h}"
                )
                for j in range(JT):
                    ps = smallp_pool.tile([128, 128], f32, tag="small", name="smallps")
                    nc.tensor.matmul(
                        ps[:],
                        lhsT=hx_tile[:, 128 * j : 128 * j + 128],
                        rhs=wbd_sb[h][:],
                        start=True,
                        stop=True,
                    )
                    nc.vector.tensor_copy(stg[:, j, :], ps[:])
                civ = cc_in[(t, h)].rearrange("(j p) c -> p j c", p=128)
                nc.sync.dma_start(civ[:, :, :], stg[:])
                nc.gpsimd.collective_compute(
                    "AllGather",
                    mybir.AluOpType.bypass,
                    replica_groups=groups,
                    ins=[cc_in[(t, h)][:, :]],
                    outs=[cc_out[(t, h)][:, :]],
                )
                cov = cc_out[(t, h)].rearrange("(i k p) c -> p (i k) c", p=128, k=8)
                for i in range(8):
                    nc.sync.dma_start(linf[h][i][:], cov[:, 8 * i : 8 * i + 8, :])

            def big_mm(t, m, dst_tiles):
                """hxT_new[gh-half m] = lin_full.T @ LT + wiu, tanh."""
                for nh in range(2):
                    ps = bigp_pool.tile([128, 512], f32, tag="big", name="bigps")
                    sl = slice(512 * nh, 512 * nh + 512)
                    for k in range(KT):
                        nc.tensor.matmul(
                            ps[:],
                            lhsT=linf[m][k // 8][:, k % 8, :],
                            rhs=lt_slice(k, nh),
                            start=(k == 0),
                            stop=(k == KT - 1),
                        )
                    nc.vector.tensor_add(ps[:], ps[:], wiu_sb[m][:, sl])
                    nc.scalar.activation(dst_tiles[m][:, sl], ps[:], Tanh)

            # ---- software-pipelined iterations 1..4 ----
            # PE order: M0(t) | smallA(t+1)+AG_A | M1(t) | smallB(t+1)+AG_B
            # so each AllGather hides under ~27us of the other half's pass.
            hxt = None
            for t in range(1, NITER):
                last = t == NITER - 1
                if last:
                    nxt = [
                        out_pool.tile([128, ROWS], f32, tag=f"o{m}", name=f"o{m}")
                        for m in range(2)
                    ]
                else:
                    nxt = [
                        hxt_pool.tile([128, ROWS], bf16, tag="hxt", name="hxt")
                        for _ in range(2)
                    ]
                big_mm(t, 0, nxt)
                if not last:
                    small_mm_and_ag(t + 1, 0, nxt[0])
                big_mm(t, 1, nxt)
                if not last:
                    small_mm_and_ag(t + 1, 1, nxt[1])
                hxt = nxt

            for m in range(2):
                nc.sync.dma_start(out_d[128 * m : 128 * m + 128, :], hxt[m][:])

    nc.compile()
    return nc


def _prep_inputs(X, L, W_ih, W_hh):
    bf = ml_dtypes.bfloat16
    Lb = np.ascontiguousarray(L.T).astype(bf)  # [N, N] transposed, bf16
    XT = np.ascontiguousarray(X.T)  # [F, N] f32
    XTFb = XT.astype(bf)
    WihT = np.ascontiguousarray(W_ih.reshape(GH, F).T)  # [F, GH]
    wbd = [np.zeros((128, 128), np.float32) for _ in range(2)]
    for g in range(G):
        h = g // 2
        o = (g % 2) * H
        wbd[h][o : o + H, o : o + H] = W_hh[g].T
    in_maps = []
    for c in range(NCORES):
        sl = slice(c * ROWS, (c + 1) * ROWS)
        in_maps.append(
            {
                "LT": np.ascontiguousarray(Lb[:, sl]),
                "XT": np.ascontiguousarray(XT[:, sl]),
                "XTF": XTFb,
                "WihT": WihT,
                "WihTb": WihT.astype(bf),
                "Wbd0": wbd[0].astype(bf),
                "Wbd1": wbd[1].astype(bf),
            }
        )
    return in_maps


def kernel(X, L, W_ih, W_hh, trace=False):
    from concourse.bass_utils import run_bass_kernel_spmd

    X = np.asarray(X, np.float32)
    L = np.asarray(L, np.float32)
    W_ih = np.asarray(W_ih, np.float32)
    W_hh = np.asarray(W_hh, np.float32)

    if "nc" not in _CACHE:
        _CACHE["nc"] = _build_kernel()
    in_maps = _prep_inputs(X, L, W_ih, W_hh)
    res = run_bass_kernel_spmd(
        _CACHE["nc"], in_maps, list(range(NCORES)), trace=trace
    )
    out = np.empty((N, GH), np.float32)
    for c in range(NCORES):
        out[c * ROWS : (c + 1) * ROWS, :] = res.results[c]["hxT_out"].T
    _CACHE["last_result"] = res
    return out



# revision 10
# speedup vs baseline: 1.1480x; 1.1480x over previous
"""GroupOfGESNCell Trainium2 kernel.

Math (reference): 5 fixed-point iterations over G=4 groups:
    wiu = einsum('nf,ghf->gnh', X, W_ih)                     # [G,N,H]
    hx  <- tanh(wiu + L @ (hx @ W_hh_g^T))   per group       # N=8192, H=64
    out = concat_g(hx_g) -> [N, G*H=256]
The convergence early-exit (diff < 1e-5) never triggers for this input
regime (diffs stay O(100)); 5 unconditional iterations are exact.
Iteration 0 starts from hx=0, so it reduces to hx1 = tanh(wiu) with no
L-matmul: only 4 big matmuls of L are needed.

Distribution: row-shard L over 8 cores (1024 rows each). Each core
keeps its shard of L resident in SBUF (bf16, 16 MB) for ALL
iterations -- L is loaded from HBM exactly once. Per iteration each
core computes lin = hx @ Whh^T for its rows, AllGathers lin (bf16,
0.25 MB/rank, split in two gh-halves to pipeline against the PE),
then computes new-hx rows = L_rows @ lin_full + wiu, tanh.

Iteration 1 is collective-free: hx1 = tanh(X @ Wih^T) depends only on
X, so every core redundantly computes lin_1 for ALL nodes from the
full X (bf16). This hides the ~50us first-collective/ncfw warmup and
the kernel-entry barrier under iteration-1 compute; the first real
AllGather (for iteration 2) enters the steady-state pipeline.

On-device layout: hx is kept transposed (hxT [GH, n]) so every matmul
consumes natural layouts with zero on-device transposes:
  small mm: lin[n,gk] = hxT[gh,n].T @ blockdiag(WhhT)    (lhsT = hxT)
  big   mm: hxT_new[gh,n] = lin_full[m,gh].T @ LT[m,n]   (lhsT = lin)
The host pre-transposes L (per-shard) and X once; that is sharding
prep, not part of the 172-GFLOP iteration.
"""

import sys

import numpy as np
import ml_dtypes

sys.path.insert(0, "/opt/trn_rl_repo")

N, F, H, G = 8192, 128, 64, 4
GH = G * H  # 256
NCORES = 8
ROWS = N // NCORES  # 1024 rows of L / nodes per core
KT = N // 128  # 64 contraction tiles for the big matmul
JT = ROWS // 128  # 8 n-tiles per core
NITER = 5

_CACHE = {}


def _build_kernel():
    import concourse.mybir as mybir
    import concourse.tile as tile
    from concourse import bacc

    f32 = mybir.dt.float32
    bf16 = mybir.dt.bfloat16
    Tanh = mybir.ActivationFunctionType.Tanh

    nc = bacc.Bacc(
        "TRN2", target_bir_lowering=False, debug=False, num_devices=NCORES
    )

    # Per-core inputs (host-prepped):
    #  LT    [N, ROWS] bf16 : L[rows_c, :].T  (contraction dim first)
    #  XT    [F, ROWS] f32  : X[rows_c, :].T   (for the f32 wiu kept locally)
    #  XTF   [F, N]    bf16 : full X.T         (iteration-1 redundant path)
    #  WihT  [F, GH]   f32  ; WihTb same in bf16
    #  Wbd0/1 [128,128] bf16: blockdiag(Whh_{2h}^T, Whh_{2h+1}^T)
    # Output: hxT_out [GH, ROWS] f32 (host transposes + stacks)
    lt_d = nc.declare_dram_parameter("LT", [N, ROWS], bf16, isOutput=False)
    xt_d = nc.declare_dram_parameter("XT", [F, ROWS], f32, isOutput=False)
    xtf_d = nc.declare_dram_parameter("XTF", [F, N], bf16, isOutput=False)
    wih_d = nc.declare_dram_parameter("WihT", [F, GH], f32, isOutput=False)
    wihb_d = nc.declare_dram_parameter("WihTb", [F, GH], bf16, isOutput=False)
    wbd_d = [
        nc.declare_dram_parameter(f"Wbd{h}", [128, 128], bf16, isOutput=False)
        for h in range(2)
    ]
    out_d = nc.declare_dram_parameter("hxT_out", [GH, ROWS], f32, isOutput=True)

    # Collective bounce buffers for iterations 2..4, per gh-half.
    # half=0 carries lin cols 0:128 (groups 0,1), half=1 cols 128:256.
    cc_in = {
        (t, h): nc.dram_tensor(f"ccin_{t}_{h}", [ROWS, 128], bf16)
        for t in range(2, NITER)
        for h in range(2)
    }
    cc_out = {
        (t, h): nc.dram_tensor(f"ccout_{t}_{h}", [N, 128], bf16, addr_space="Shared")
        for t in range(2, NITER)
        for h in range(2)
    }
    warm_in = nc.dram_tensor("warm_in", [128, 16], mybir.dt.bfloat16)
    warm_out = nc.dram_tensor(
        "warm_out", [128 * NCORES, 16], mybir.dt.bfloat16, addr_space="Shared"
    )
    groups = [list(range(NCORES))]

    with tile.TileContext(nc) as tc:
        with (
            tc.tile_pool(name="lt", bufs=1) as lt_pool,
            tc.tile_pool(name="linf", bufs=1) as linf_pool,
            tc.tile_pool(name="hxt", bufs=2) as hxt_pool,
            tc.tile_pool(name="wiu", bufs=1) as wiu_pool,
            tc.tile_pool(name="consts", bufs=1) as const_pool,
            tc.tile_pool(name="xtf", bufs=4) as xtf_pool,
            tc.tile_pool(name="hx1f", bufs=2) as hx1f_pool,
            tc.tile_pool(name="stage", bufs=1) as stage_pool,
            tc.tile_pool(name="outs", bufs=1) as out_pool,
            tc.tile_pool(name="bigp", bufs=4, space="PSUM") as bigp_pool,
            tc.tile_pool(name="smallp", bufs=4, space="PSUM") as smallp_pool,
        ):
            # ---- constants / static loads (sync queue) ----
            # xt/wih first: the wiu matmuls are the first PE work
            xt_sb = const_pool.tile([F, ROWS], f32, tag="xt")
            nc.sync.dma_start(xt_sb[:], xt_d[:, :])
            wih_sb = const_pool.tile([F, GH], f32, tag="wih")
            nc.sync.dma_start(wih_sb[:], wih_d[:, :])
            wihb_sb = const_pool.tile([F, GH], bf16, tag="wihb")
            nc.sync.dma_start(wihb_sb[:], wihb_d[:, :])
            wbd_sb = [
                const_pool.tile([128, 128], bf16, tag=f"wbd{h}", name=f"wbd{h}")
                for h in range(2)
            ]
            for h in range(2):
                nc.sync.dma_start(wbd_sb[h][:], wbd_d[h][:, :])
            # dummy AllGather, hidden under the prologue: absorbs the
            # ncfw/SDMA warm-up so the first real AllGather runs at the
            # steady ~16us instead of ~35us.
            nc.gpsimd.collective_compute(
                "AllGather",
                mybir.AluOpType.bypass,
                replica_groups=groups,
                ins=[warm_in[:, :]],
                outs=[warm_out[:, :]],
            )
            # ALL X.T chunks loaded BEFORE the LT triggers: anything queued
            # after LT waits ~47us for 16 MB of L, which stalled the whole
            # iteration-1 lin pipeline (chunks 2-3 used to be issued late).
            xtf_tiles = []
            for cc in range(4):
                t_ = xtf_pool.tile([128, 2048], bf16, tag="xtfc", name="xtfc")
                nc.sync.dma_start(t_[:], xtf_d[:, 2048 * cc : 2048 * cc + 2048])
                xtf_tiles.append(t_)

            # L-shard resident in SBUF: 16 tiles of [128, 4, ROWS] bf16
            # (tile i holds contraction k-tiles 4i..4i+3). Triggered on the
            # sync queue (nothing latency-critical queues behind them in the
            # prologue); the ACT queue stays free for the tanh chain.
            lt_view = lt_d.rearrange("(i k p) n -> p (i k) n", p=128, k=4)
            lt_sb = []
            for i in range(16):
                t_ = lt_pool.tile([128, 4, ROWS], bf16, tag=f"lt{i}", name=f"lt{i}")
                nc.sync.dma_start(t_[:], lt_view[:, 4 * i : 4 * i + 4, :])
                lt_sb.append(t_)

            def lt_slice(k, nh):
                return lt_sb[k // 4][:, k % 4, 512 * nh : 512 * nh + 512]

            # ---- local wiu = (X_c @ Wih^T)^T in f32, for the per-iter add
            wiu_sb = [
                wiu_pool.tile([128, ROWS], f32, tag=f"wiu{m}", name=f"wiu{m}")
                for m in range(2)
            ]
            for m in range(2):
                for nh in range(2):
                    ps = bigp_pool.tile([128, 512], f32, tag="big", name="bigps")
                    nc.tensor.matmul(
                        ps[:],
                        lhsT=wih_sb[:, 128 * m : 128 * m + 128],
                        rhs=xt_sb[:, 512 * nh : 512 * nh + 512],
                        start=True,
                        stop=True,
                    )
                    nc.scalar.copy(wiu_sb[m][:, 512 * nh : 512 * nh + 512], ps[:])

            # gathered lin, bf16: 8 tiles of [128, 8, 128] per gh-half
            # (one tile per DMA so each k-matmul waits on a single queue)
            linf = [
                [
                    linf_pool.tile(
                        [128, 8, 128], bf16, tag=f"linf{h}_{i}", name=f"linf{h}_{i}"
                    )
                    for i in range(8)
                ]
                for h in range(2)
            ]

            # ---- iteration-1 lin for ALL nodes, computed locally ----
            # hx1 = tanh(X @ Wih^T); lin1 = hx1 @ blockdiag -> linf direct.
            for cc in range(4):  # [128, 2048] chunks of full X.T
                xtfc = xtf_tiles[cc]
                for m in range(2):
                    for s in range(4):  # 512-col sub-chunks
                        c = 4 * cc + s  # global 512-chunk 0..15
                        ps = bigp_pool.tile([128, 512], f32, tag="big", name="bigps")
                        nc.tensor.matmul(
                            ps[:],
                            lhsT=wihb_sb[:, 128 * m : 128 * m + 128],
                            rhs=xtfc[:, 512 * s : 512 * s + 512],
                            start=True,
                            stop=True,
                        )
                        hx1c = hx1f_pool.tile([128, 512], bf16, tag="hx1c", name="hx1c")
                        nc.scalar.activation(hx1c[:], ps[:], Tanh)
                        for j in range(4):
                            k = 4 * c + j  # global k-tile 0..63
                            ps2 = smallp_pool.tile(
                                [128, 128], f32, tag="small", name="smallps"
                            )
                            nc.tensor.matmul(
                                ps2[:],
                                lhsT=hx1c[:, 128 * j : 128 * j + 128],
                                rhs=wbd_sb[m][:],
                                start=True,
                                stop=True,
                            )
                            nc.vector.tensor_copy(linf[m][k // 8][:, k % 8, :], ps2[:])

            def small_mm_and_ag(t, h, hx_tile):
                """lin cols [128h:128h+128] for local rows from hx_tile,
                then AllGather into linf[h].

                The stage->cc_in stores ride the gpsimd queue (right before
                the collective trigger, which also lives there) so they are
                never stuck behind the ~18us of linf loads on the sync
                queue -- that queueing used to delay every AllGather by
                ~10us and exposed it in the big-mm stream."""
                stg = stage_pool.tile(
                    [128, JT, 128], bf16, tag=f"stg{h}", name=f"stg{h}"
                )
                civ = cc_in[(t, h)].rearrange("(j p) c -> p j c", p=128)
                for half in range(2):
                    for j in range(JT // 2 * half, JT // 2 * (half + 1)):
                        ps = smallp_pool.tile(
                            [128, 128], f32, tag="small", name="smallps"
                        )
                        nc.tensor.matmul(
                            ps[:],
                            lhsT=hx_tile[:, 128 * j : 128 * j + 128],
                            rhs=wbd_sb[h][:],
                            start=True,
                            stop=True,
                        )
                        nc.vector.tensor_copy(stg[:, j, :], ps[:])
                    j0, j1 = JT // 2 * half, JT // 2 * (half + 1)
                    nc.gpsimd.dma_start(civ[:, j0:j1, :], stg[:, j0:j1, :])
                nc.gpsimd.collective_compute(
                    "AllGather",
                    mybir.AluOpType.bypass,
                    replica_groups=groups,
                    ins=[cc_in[(t, h)][:, :]],
                    outs=[cc_out[(t, h)][:, :]],
                )
                cov = cc_out[(t, h)].rearrange("(i k p) c -> p (i k) c", p=128, k=8)
                # first k-tiles split out so the consuming matmul can start
                # ~1.7us earlier than a monolithic 8-k-tile load allows
                nc.sync.dma_start(linf[h][0][:, 0:2, :], cov[:, 0:2, :])
                nc.sync.dma_start(linf[h][0][:, 2:8, :], cov[:, 2:8, :])
                for i in range(1, 8):
                    nc.sync.dma_start(linf[h][i][:], cov[:, 8 * i : 8 * i + 8, :])

            def big_mm(t, m, dst_tiles, last=False):
                """hxT_new[gh-half m] = lin_full.T @ LT + wiu, tanh."""
                for nh in range(2):
                    ps = bigp_pool.tile([128, 512], f32, tag="big", name="bigps")
                    sl = slice(512 * nh, 512 * nh + 512)
                    for k in range(KT):
                        nc.tensor.matmul(
                            ps[:],
                            lhsT=linf[m][k // 8][:, k % 8, :],
                            rhs=lt_slice(k, nh),
                            start=(k == 0),
                            stop=(k == KT - 1),
                        )
                    nc.vector.tensor_add(ps[:], ps[:], wiu_sb[m][:, sl])
                    if last:
                        # [128,512] f32 staging chunk + immediate store: the
                        # DMA overlaps the remaining tanh work
                        oc = out_pool.tile([128, 512], f32, tag=f"oc{m}", name="oc")
                        nc.scalar.activation(oc[:], ps[:], Tanh)
                        nc.sync.dma_start(out_d[128 * m : 128 * m + 128, sl], oc[:])
                    else:
                        nc.scalar.activation(dst_tiles[m][:, sl], ps[:], Tanh)

            # ---- software-pipelined iterations 1..4 ----
            # PE order: M0(t) | smallA(t+1)+AG_A | M1(t) | smallB(t+1)+AG_B
            # so each AllGather hides under ~27us of the other half's pass.
            hxt = None
            for t in range(1, NITER):
                last = t == NITER - 1
                if last:
                    nxt = [None, None]
                else:
                    nxt = [
                        hxt_pool.tile([128, ROWS], bf16, tag="hxt", name="hxt")
                        for _ in range(2)
                    ]
                big_mm(t, 0, nxt, last=last)
                if not last:
                    small_mm_and_ag(t + 1, 0, nxt[0])
                big_mm(t, 1, nxt, last=last)
                if not last:
                    small_mm_and_ag(t + 1, 1, nxt[1])
                hxt = nxt

    nc.compile()
    return nc


def _prep_inputs(X, L, W_ih, W_hh):
    bf = ml_dtypes.bfloat16
    Lb = np.ascontiguousarray(L.T).astype(bf)  # [N, N] transposed, bf16
    XT = np.ascontiguousarray(X.T)  # [F, N] f32
    XTFb = XT.astype(bf)
    WihT = np.ascontiguousarray(W_ih.reshape(GH, F).T)  # [F, GH]
    wbd = [np.zeros((128, 128), np.float32) for _ in range(2)]
    for g in range(G):
        h = g // 2
        o = (g % 2) * H
        wbd[h][o : o + H, o : o + H] = W_hh[g].T
    in_maps = []
    for c in range(NCORES):
        sl = slice(c * ROWS, (c + 1) * ROWS)
        in_maps.append(
            {
                "LT": np.ascontiguousarray(Lb[:, sl]),
                "XT": np.ascontiguousarray(XT[:, sl]),
                "XTF": XTFb,
                "WihT": WihT,
                "WihTb": WihT.astype(bf),
                "Wbd0": wbd[0].astype(bf),
                "Wbd1": wbd[1].astype(bf),
            }
        )
    return in_maps


def kernel(X, L, W_ih, W_hh, trace=False):
    from concourse.bass_utils import run_bass_kernel_spmd

    X = np.asarray(X, np.float32)
    L = np.asarray(L, np.float32)
    W_ih = np.asarray(W_ih, np.float32)
    W_hh = np.asarray(W_hh, np.float32)

    if "nc" not in _CACHE:
        _CACHE["nc"] = _build_kernel()
    in_maps = _prep_inputs(X, L, W_ih, W_hh)
    res = run_bass_kernel_spmd(
        _CACHE["nc"], in_maps, list(range(NCORES)), trace=trace
    )
    out = np.empty((N, GH), np.float32)
    for c in range(NCORES):
        out[c * ROWS : (c + 1) * ROWS, :] = res.results[c]["hxT_out"].T
    _CACHE["last_result"] = res
    return out

